# revision 4
# baseline (speedup 1.0000x reference)
"""Class-based decoder (MoE-style routing) on 8 trn2 NeuronCores.

Strategy: expert-parallel. Classes are padded 250->256 and split 32 per core.
On the host, tokens are grouped by class into capacity-padded slots (C tokens
per class slot, C in {32,64,128}); class slots that overflow C spill into
extra slots holding a duplicate of the class weights.  Each core receives:
  - xT   [128, n_mt*KCH*128]  its padded tokens, pre-transposed k-major
  - wcT  [128, KCH*NCLS]      the (replicated) class-decoder weights, k-major
  - wwT  [slots, 128, KCH*CHUNK]  its word-decoder weight shard, k-major
and computes, for every 128-token tile, the class logits (x @ Wc.T) and the
per-class word logits (x_c @ Ww[c].T) as fp32 PE matmuls accumulating K=512
over 4 PSUM chunks; the 128//C class slots of a tile are col-tiled into one
PSUM tile.  Biases (zero in practice, but handled for correctness) are added
on the host during the final unpermute.
"""

import numpy as np
from contextlib import ExitStack

import concourse.bass as bass
import concourse.bacc as bacc
import concourse.tile as tile
import concourse.mybir as mybir
from concourse.bass_utils import run_bass_kernel_spmd

NHID = 512
NCLS = 250
CHUNK = 200
NCORES = 8
KCH = NHID // 128          # 4 contraction chunks of 128
NCLS_PAD = 256             # classes padded so each core owns an equal shard
CPC = NCLS_PAD // NCORES   # classes per core
NCOL = NCLS + CHUNK        # 450 output columns
F32 = mybir.dt.float32

LAST_RESULT = None         # BassKernelResults of the most recent device run
_program_cache = {}


def _build_program(C, slots):
    """One SPMD program: slots class-slots of C tokens each, per core."""
    per_mt = 128 // C          # class slots per 128-token m-tile
    n_mt = (slots * C) // 128  # 128-token m-tiles
    npad = slots * C

    nc = bacc.Bacc("TRN2", target_bir_lowering=False, debug=False,
                   num_devices=NCORES)
    xT = nc.dram_tensor("xT", [128, n_mt * KCH * 128], F32, kind="ExternalInput")
    wcT = nc.dram_tensor("wcT", [128, KCH * NCLS], F32, kind="ExternalInput")
    wwT = nc.dram_tensor("wwT", [slots, 128, KCH * CHUNK], F32,
                         kind="ExternalInput")
    out = nc.dram_tensor("out", [npad, NCOL], F32, kind="ExternalOutput")

    with tile.TileContext(nc) as tc, ExitStack() as ctx:
        xpool = ctx.enter_context(tc.tile_pool(name="x", bufs=1))
        wcpool = ctx.enter_context(tc.tile_pool(name="wc", bufs=1))
        wpool = ctx.enter_context(tc.tile_pool(name="w", bufs=8))
        opool = ctx.enter_context(tc.tile_pool(name="o", bufs=3))
        pcp = ctx.enter_context(
            tc.tile_pool(name="pc", bufs=2, space=bass.MemorySpace.PSUM))
        pwp = ctx.enter_context(
            tc.tile_pool(name="pw", bufs=2, space=bass.MemorySpace.PSUM))

        wc_sb = wcpool.tile([128, KCH * NCLS], F32)
        nc.sync.dma_start(wc_sb[:], wcT[:])
        x_sb = xpool.tile([128, n_mt * KCH * 128], F32)

        for m in range(n_mt):
            # x columns for this m-tile: [(m*KCH+j)*128 + t] for j in 0..KCH
            nc.sync.dma_start(x_sb[:, m * KCH * 128:(m + 1) * KCH * 128],
                              xT[:, m * KCH * 128:(m + 1) * KCH * 128])

            def xcol(j, lo, hi):
                base = (m * KCH + j) * 128
                return x_sb[:, base + lo:base + hi]

            # class logits for these 128 tokens: [128, NCLS]
            pc_ps = pcp.tile([128, NCLS], F32)
            for j in range(KCH):
                nc.tensor.matmul(
                    pc_ps[:, :],
                    xcol(j, 0, 128),
                    wc_sb[:, j * NCLS:(j + 1) * NCLS],
                    start=(j == 0), stop=(j == KCH - 1),
                )

            # word logits: per_mt class slots col-tiled into one [128, CHUNK]
            pw_ps = pwp.tile([128, CHUNK], F32)
            for q in range(per_mt):
                s = m * per_mt + q
                w_sb = wpool.tile([128, KCH * CHUNK], F32, tag="w")
                nc.sync.dma_start(w_sb[:], wwT[s])
                for j in range(KCH):
                    nc.tensor.matmul(
                        pw_ps[q * C:(q + 1) * C, :],
                        xcol(j, q * C, (q + 1) * C),
                        w_sb[:, j * CHUNK:(j + 1) * CHUNK],
                        start=(j == 0), stop=(j == KCH - 1),
                        tile_position=(0, q * C),
                    )

            o_sb = opool.tile([128, NCOL], F32)
            nc.vector.tensor_copy(o_sb[:, :NCLS], pc_ps[:])
            nc.vector.tensor_copy(o_sb[:, NCLS:], pw_ps[:])
            nc.sync.dma_start(out[m * 128:(m + 1) * 128, :], o_sb[:])

    nc.compile()
    return nc


def _route(cls):
    """Group tokens by class into capacity-padded slots.

    Returns (C, slots, tok_idx [NCORES, slots*C] int64 token id or -1,
    slot_cls [NCORES, slots] class id per slot or -1 for dummy slots).
    """
    counts = np.bincount(cls, minlength=NCLS_PAD)
    cmax = int(counts.max())
    C = 32
    while C < cmax and C < 128:
        C *= 2

    order = np.argsort(cls, kind="stable")
    starts = np.zeros(NCLS_PAD + 1, np.int64)
    starts[1:] = np.cumsum(counts)

    jobs = [[] for _ in range(NCORES)]  # (class, tok_start, tok_count)
    for k in range(NCORES):
        for c in range(k * CPC, (k + 1) * CPC):
            cnt = int(counts[c])
            off = 0
            while True:
                jobs[k].append((c, int(starts[c]) + off, min(C, cnt - off)))
                off += C
                if off >= cnt:
                    break
    per_mt = 128 // C
    slots = max(len(j) for j in jobs)
    slots = -(-slots // per_mt) * per_mt  # round up to multiple of per_mt

    tok_idx = np.full((NCORES, slots * C), -1, np.int64)
    slot_cls = np.full((NCORES, slots), -1, np.int64)
    for k in range(NCORES):
        for s, (c, lo, n) in enumerate(jobs[k]):
            slot_cls[k, s] = c
            if n > 0:
                tok_idx[k, s * C:s * C + n] = order[lo:lo + n]
    return C, slots, tok_idx, slot_cls


def kernel(x, Wc, bc, Ww, bw, cls_idx, _trace=False, _trace_cores=None):
    global LAST_RESULT
    x = np.ascontiguousarray(np.asarray(x, np.float32))
    Wc = np.ascontiguousarray(np.asarray(Wc, np.float32))
    bc = np.asarray(bc, np.float32)
    Ww = np.ascontiguousarray(np.asarray(Ww, np.float32))
    bw = np.asarray(bw, np.float32)
    cls = np.asarray(cls_idx).astype(np.int64).ravel()
    N = cls.shape[0]

    C, slots, tok_idx, slot_cls = _route(cls)
    npad = slots * C
    n_mt = npad // 128

    key = (C, slots)
    if key not in _program_cache:
        _program_cache[key] = _build_program(C, slots)
    nc = _program_cache[key]

    # wcT [128, KCH*NCLS]: wcT[p, j*NCLS+c] = Wc[c, j*128+p]  (replicated)
    wcT = np.ascontiguousarray(
        Wc.reshape(NCLS, KCH, 128).transpose(2, 1, 0).reshape(128, KCH * NCLS))

    # per-core word-decoder shard, slot order, zero for dummy slots
    Ww_pad = np.zeros((NCLS_PAD, CHUNK, NHID), np.float32)
    Ww_pad[:NCLS] = Ww

    in_maps = []
    for k in range(NCORES):
        # wwT[s, p, j*CHUNK+w] = Ww[slot_cls[k,s], w, j*128+p]
        wsel = Ww_pad[np.maximum(slot_cls[k], 0)]
        wsel[slot_cls[k] < 0] = 0.0
        wwT = np.ascontiguousarray(
            wsel.reshape(slots, CHUNK, KCH, 128)
                .transpose(0, 3, 2, 1).reshape(slots, 128, KCH * CHUNK))

        ti = tok_idx[k]
        xk = x[np.maximum(ti, 0)]
        xk[ti < 0] = 0.0
        # xT[p, (m*KCH+j)*128 + t] = xk[m*128+t, j*128+p]
        xT = np.ascontiguousarray(
            xk.reshape(n_mt, 128, KCH, 128).transpose(3, 0, 2, 1)
              .reshape(128, n_mt * KCH * 128))
        in_maps.append({"xT": xT, "wcT": wcT, "wwT": wwT})

    LAST_RESULT = run_bass_kernel_spmd(
        nc, in_maps, list(range(NCORES)), trace=_trace,
        trace_cores=(_trace_cores if _trace else None))

    out = np.zeros((N, NCOL), np.float32)
    for k in range(NCORES):
        ok = LAST_RESULT.results[k]["out"]
        valid = tok_idx[k] >= 0
        out[tok_idx[k][valid]] = ok[valid]

    out[:, :NCLS] += bc
    out[:, NCLS:] += bw[cls]
    return out
